# revision 26
# baseline (speedup 1.0000x reference)
"""BitStackLinear Trainium2 kernel (v2 — software-pipelined).

y = x @ w.T with w = sum_b sign_b * (u_b @ vt_b), signs bit-packed in qweight.

Strategy: column-parallel over out_features across 8 NeuronCores. Each core
builds w in bf16 (phase A) and runs y = x @ w.T (phase B), software-pipelined
so phase A for out-group og+1 runs on DVE/ACT while phase B for og runs on PE.

Phase A per 128-row i'-chunk (contraction permuted bit-plane-major so each
chunk uses one constant bit position j):
  - low-rank lr_b = 2*u_b @ vt_b via 4 concurrent K=16 row-group
    matmuls (PE), evicted PSUM->SBUF bf16 by ACT
  - DVE: AND isolates bit j as u16 {0, 2^j} (4x tensor_scalar rate), then
    one wide tensor_mul m_b = (q & 2^j) * (2^-j * 2 * lr_b) -- vtp columns
    are host-scaled by 2^-j so the power-of-2 factors cancel exactly
  - two pair-adds -> wa_chunk = sum_b m_b
The identity sign = 2*bit - 1 means w = wa - sum_b u_b@vt_b; the rank-64
correction is folded into phase B as one extra accumulating matmul per
(tq, u, og) using host-computed G = x @ vt^T (tiny) and -u.

Phase B per og: 4 token-quads, each accumulating 1 correction matmul + 32
i'-chunk matmuls into 4 PSUM banks; y evicted as bf16 (host casts to f32).
"""
import sys

for _p in ("/opt/trn_rl_repo", "/root/.axon_site/_ro/trn_rl_repo"):
    if _p not in sys.path:
        sys.path.insert(0, _p)

import numpy as np
import ml_dtypes

import concourse.bass as bass
import concourse.tile as tile
from concourse import mybir
from concourse.bass_utils import run_bass_kernel_spmd

N_CORES = 8
B = 4       # bit planes
K = 16      # low-rank
T = 2048    # tokens
I = 4096    # in_features
O = 11008   # out_features
O_S = O // N_CORES  # 1376 per core

_SLOT = 512  # psum bank free width (f32)


def _og_groups(o_s):
    """Out-feature groups, ascending width (critical-path model optimum:
    fill is priced at DVE rate on og0, tail at PE rate on the last og)."""
    if o_s == 1376:
        widths = [352, 512, 512]
    else:
        widths, rem = [], o_s
        while rem > 0:
            w = min(_SLOT, rem)
            widths.append(w)
            rem -= w
        widths.sort()
    out, o0 = [], 0
    for w in widths:
        out.append((o0, w))
        o0 += w
    return out


def build_nc(t=T, i=I, o_s=O_S):
    """Build the per-core SPMD Bass program (identical on all cores)."""
    nb = i // 8          # packed words per (b, o)
    mb_n = nb // 128     # byte-row blocks per bit plane
    nc_i = i // 128      # i'-chunks
    assert nc_i == 8 * mb_n and t % 512 == 0
    n_tq = t // 512      # token quads (4 chunks of 128 per quad)
    ogs = _og_groups(o_s)
    n_og = len(ogs)

    # phase A production order = phase B consumption order
    c_order = [j * mb_n + mb for mb in range(mb_n) for j in range(8)]

    nc = bass.Bass("TRN2", target_bir_lowering=False, debug=False)

    xt_d = nc.dram_tensor("xt", [i, t], mybir.dt.bfloat16, kind="ExternalInput")
    qt_d = nc.dram_tensor("qt", [B, nb, o_s], mybir.dt.uint16, kind="ExternalInput")
    vt_d = nc.dram_tensor("vtp", [128, i], mybir.dt.bfloat16, kind="ExternalInput")
    ut_d = nc.dram_tensor("utp", [128, o_s], mybir.dt.bfloat16, kind="ExternalInput")
    gt_d = nc.dram_tensor("gt", [64, t], mybir.dt.bfloat16, kind="ExternalInput")
    un_d = nc.dram_tensor("un", [64, o_s], mybir.dt.bfloat16, kind="ExternalInput")
    y_d = nc.dram_tensor("y", [t, o_s], mybir.dt.bfloat16, kind="ExternalOutput")

    f32 = mybir.dt.float32
    bf16 = mybir.dt.bfloat16
    u16 = mybir.dt.uint16
    AND = mybir.AluOpType.bitwise_and
    MULT = mybir.AluOpType.mult

    with tile.TileContext(nc) as tc:
        with (
            tc.tile_pool(name="const", bufs=1) as cpool,
            tc.tile_pool(name="w", bufs=1) as wpool,
            tc.tile_pool(name="q", bufs=2) as qpool,
            tc.tile_pool(name="andt", bufs=4) as andpool,
            tc.tile_pool(name="lrsb", bufs=5) as lrsbpool,
            tc.tile_pool(name="m", bufs=4) as mpool,
            tc.tile_pool(name="t", bufs=5) as tpool,
            tc.tile_pool(name="x", bufs=12) as xpool,
            tc.tile_pool(name="ysb", bufs=4) as ysbpool,
            tc.tile_pool(name="lrps", bufs=1, space="PSUM") as lrps,
            tc.tile_pool(name="yps", bufs=1, space="PSUM") as yps,
        ):
            # ---- persistent loads (q[0]/vt/ut first: phase A needs them
            # immediately; gt/un only gate the first phase-B matmul) ----
            q_sb = []

            def load_q(mb, eng=None):
                q_t = qpool.tile([128, B * o_s], u16, name=f"q{mb}",
                                 tag=f"q{mb % 2}")
                for b in range(B):
                    (eng or nc.sync).dma_start(
                        q_t[:, b * o_s:(b + 1) * o_s],
                        qt_d.ap()[b, mb * 128:(mb + 1) * 128, :],
                    )
                q_sb.append(q_t)

            # preamble loads split across engine DMA queues so q0 (gates
            # the first AND), vt/ut (gate the first lr matmul) and gt/un
            # (gate the first correction matmul) land in parallel
            load_q(0)
            vt_sb = cpool.tile([128, i], bf16, tag="vt")
            nc.scalar.dma_start(vt_sb[:, :512], vt_d.ap()[:, :512])
            ut_sb = cpool.tile([128, o_s], bf16, tag="ut")
            nc.scalar.dma_start(ut_sb[:], ut_d.ap())
            nc.scalar.dma_start(vt_sb[:, 512:], vt_d.ap()[:, 512:])
            gt_sb = cpool.tile([64, t], bf16, tag="gt")
            nc.gpsimd.dma_start(gt_sb[:], gt_d.ap())
            un_sb = cpool.tile([64, o_s], bf16, tag="un")
            nc.gpsimd.dma_start(un_sb[:], un_d.ap())
            for mb in range(1, mb_n):
                load_q(mb)

            w_tiles = {}

            def make_w(ogi):
                _, ow = ogs[ogi]
                w_tiles[ogi] = wpool.tile(
                    [128, nc_i * ow], bf16, name=f"w{ogi}", tag=f"w{ogi % 2}")

            def emit_a_chunk(ogi, ci):
                """Produce w_tiles[ogi][:, ci*ow:(ci+1)*ow]."""
                o0, ow = ogs[ogi]
                c = c_order[ci]
                j, mb = c // mb_n, c % mb_n
                q_t = q_sb[mb]
                # low-rank (pre-scaled): 4 concurrent K=16 row-group matmuls
                lr_ps = lrps.tile([128, B * _SLOT], f32, name="lr_ps",
                                  tag="lr_ps")
                for b in range(B):
                    nc.tensor.matmul(
                        lr_ps[:, b * _SLOT:b * _SLOT + ow],
                        vt_sb[32 * b:32 * b + K, c * 128:(c + 1) * 128],
                        ut_sb[32 * b:32 * b + K, o0:o0 + ow],
                        start=True, stop=True, tile_position=(32 * b, 0),
                    )
                # evict to packed [128, B*ow] bf16
                lr_sb = lrsbpool.tile([128, B * _SLOT], bf16, tag="lrsb")
                if ow == _SLOT:
                    nc.scalar.copy(lr_sb[:], lr_ps[:])
                else:
                    nc.scalar.copy(
                        lr_sb[:, :B * ow].rearrange("p (b w) -> p b w", b=B),
                        lr_ps[:].rearrange("p (b w) -> p b w", b=B)[:, :, :ow],
                    )
                # bit j isolated as u16 {0, 2^j} (walrus rejects mixing
                # bitwise and arith ops in one instruction, and the bitwise
                # path cannot cast dtypes; the 2^j cancels against the
                # host-side 2^-j column scaling of vtp in the multiply below,
                # which converts the u16 operand to float on its arith path)
                and_t = andpool.tile([128, B * _SLOT], u16, tag="andt")
                nc.vector.tensor_scalar(
                    and_t[:, :B * ow].rearrange("p (b w) -> p b w", b=B),
                    q_t[:].rearrange("p (b w) -> p b w", b=B)[:, :, o0:o0 + ow],
                    1 << j, None, AND,
                )
                # m_b = (q_b & 2^j) * (2^-j * 2 * lr_b) — plain 2x-rate mult
                m_t = mpool.tile([128, B * _SLOT], bf16, tag="m")
                nc.vector.tensor_mul(
                    m_t[:, :B * ow], and_t[:, :B * ow], lr_sb[:, :B * ow])
                # pair-sum the 4 planes
                t_t = tpool.tile([128, 2 * _SLOT], bf16, tag="t")
                nc.vector.tensor_add(
                    t_t[:, :2 * ow], m_t[:, :2 * ow], m_t[:, 2 * ow:4 * ow])
                nc.vector.tensor_add(
                    w_tiles[ogi][:, ci * ow:(ci + 1) * ow],
                    t_t[:, :ow], t_t[:, ow:2 * ow])

            def alloc_ysums(ogi, tq):
                o0, ow = ogs[ogi]
                ysums = [
                    yps.tile([128, _SLOT], f32, name=f"ysum{u}", tag=f"y{u}")
                    for u in range(4)
                ]
                # rank-64 sign-offset correction: y -= x @ (sum_b lr_b)^T
                for u in range(4):
                    nc.tensor.matmul(
                        ysums[u][:, :ow],
                        gt_sb[0:64, tq * 512 + u * 128:
                              tq * 512 + (u + 1) * 128],
                        un_sb[0:64, o0:o0 + ow],
                        start=True, stop=False,
                    )
                return ysums

            def emit_b_iter(ogi, tq, ci, ysums):
                o0, ow = ogs[ogi]
                c = c_order[ci]
                xt_t = xpool.tile([128, 512], bf16, tag="x")
                nc.sync.dma_start(
                    xt_t[:],
                    xt_d.ap()[c * 128:(c + 1) * 128,
                              tq * 512:(tq + 1) * 512],
                )
                for u in range(4):
                    nc.tensor.matmul(
                        ysums[u][:, :ow],
                        xt_t[:, u * 128:(u + 1) * 128],
                        w_tiles[ogi][:, ci * ow:(ci + 1) * ow],
                        start=False, stop=(ci == nc_i - 1),
                    )

            def evict_y(ogi, tq, ysums):
                o0, ow = ogs[ogi]
                for u in range(4):
                    y_sb = ysbpool.tile([128, _SLOT], bf16, tag="ysb")
                    nc.scalar.copy(y_sb[:, :ow], ysums[u][:, :ow])
                    nc.sync.dma_start(
                        y_d.ap()[(tq * 4 + u) * 128:
                                 (tq * 4 + u + 1) * 128, o0:o0 + ow],
                        y_sb[:, :ow],
                    )

            def emit_b(ogi, next_ogi, skip_tq0=False):
                """y[:, og] matmuls; interleave phase A for next_ogi."""
                ow = ogs[ogi][1]
                a_next = 0
                it = 0
                # pace A-chunk injection to the DVE chunk cadence (~2.9us)
                # so injected lr matmuls never head-block the in-order PE
                # queue: wider B iterations need fewer between chunks
                arate = 3 if ow >= 450 else 4
                for tq in range(1 if skip_tq0 else 0, n_tq):
                    ysums = alloc_ysums(ogi, tq)
                    for ci in range(nc_i):
                        emit_b_iter(ogi, tq, ci, ysums)
                        it += 1
                        if next_ogi is not None and it % arate == 0 \
                                and a_next < nc_i:
                            emit_a_chunk(next_ogi, a_next)
                            a_next += 1
                    evict_y(ogi, tq, ysums)
                # drain any unemitted A chunks (safety)
                while next_ogi is not None and a_next < nc_i:
                    emit_a_chunk(next_ogi, a_next)
                    a_next += 1

            # ---- pipelined schedule ----
            make_w(0)
            for ci in range(nc_i):
                emit_a_chunk(0, ci)
            for ogi in range(n_og):
                nxt = ogi + 1 if ogi + 1 < n_og else None
                if nxt is not None:
                    make_w(nxt)
                emit_b(ogi, nxt)

    _split_waits(nc)
    return nc


def _split_waits(nc, maxw=1):
    """This walrus build rejects instructions with more than a couple of
    sync-wait commands; move excess waits onto preceding same-engine NoOps."""
    for bb in nc.m.functions[0].blocks:
        insts = bb.instructions
        idx = 0
        while idx < len(insts):
            ins = insts[idx]
            si = ins.sync_info
            if si is not None and len(si.on_wait) > maxw:
                waits = list(si.on_wait)
                extra, keep = waits[:-maxw], waits[-maxw:]
                nops = []
                for k, wt in enumerate(extra):
                    nops.append(mybir.InstNoOp(
                        name=f"{ins.name}-wsplit{k}",
                        engine=ins.engine,
                        bass_nofuse=True,
                        sync_info=mybir.SyncInfo(on_wait=[wt], on_update=[]),
                    ))
                ins.sync_info = mybir.SyncInfo(on_wait=keep,
                                               on_update=list(si.on_update))
                for k, nop in enumerate(nops):
                    nc.register_instruction(nop, overwrite=True)
                    insts.insert(idx + k, nop)
                idx += len(nops)
            idx += 1


def prep_inputs(x, qweight, u, vt, n_cores=N_CORES):
    """Host-side layout prep + sharding. Returns (in_maps, meta)."""
    t, i = x.shape
    b_, o, k_ = u.shape
    nb = i // 8
    mb_n = nb // 128
    o_s = o // n_cores

    # x -> xt[i', t] bf16 with i' = j*(i/8) + m  (j-major bit-plane order)
    xt = np.ascontiguousarray(
        x.T.reshape(nb, 8, t).transpose(1, 0, 2).reshape(i, t)
    ).astype(ml_dtypes.bfloat16)

    # qweight -> qt[b, m, o] uint16 (byte-transposed; u16 ops hit the DVE
    # 16-bit packed mode, 2x the u8 rate)
    qt = np.ascontiguousarray(
        qweight.astype(np.uint16).reshape(b_, o, nb).transpose(0, 2, 1)
    )

    # vt -> permuted into i' order, stacked into PE row groups [128, i], with
    # per-chunk 2^-j column scaling (cancelled on-chip by the {0, 2^j} AND
    # result); the sign-identity factor 2 is folded into utp.
    vtp = vt.reshape(b_, k_, nb, 8).transpose(0, 1, 3, 2).reshape(b_, k_, i)
    vt_stack = np.zeros((128, i), np.float32)
    for b in range(b_):
        vt_stack[32 * b:32 * b + k_, :] = vtp[b]
    jcol = np.arange(i) // (i // 8)  # bit position j per i' column
    vt_stack *= np.exp2(-jcol)[None, :].astype(np.float32)
    vt_stack = vt_stack.astype(ml_dtypes.bfloat16)

    # u -> 2 * u^T stacked [128, o], bf16
    ut_full = np.zeros((128, o), np.float32)
    for b in range(b_):
        ut_full[32 * b:32 * b + k_, :] = 2.0 * u[b].T
    ut_full = ut_full.astype(ml_dtypes.bfloat16)

    # G = x @ vt_b^T per plane, stacked [64, t]; un = -u^T stacked [64, o]
    gt_full = np.zeros((64, t), np.float32)
    un_full = np.zeros((64, o), np.float32)
    for b in range(b_):
        gt_full[16 * b:16 * b + k_, :] = (x @ vt[b].T).T
        un_full[16 * b:16 * b + k_, :] = -u[b].T
    gt_full = gt_full.astype(ml_dtypes.bfloat16)
    un_full = un_full.astype(ml_dtypes.bfloat16)

    in_maps = []
    for core in range(n_cores):
        o0 = core * o_s
        in_maps.append({
            "xt": xt,
            "qt": np.ascontiguousarray(qt[:, :, o0:o0 + o_s]),
            "vtp": vt_stack,
            "utp": np.ascontiguousarray(ut_full[:, o0:o0 + o_s]),
            "gt": gt_full,
            "un": np.ascontiguousarray(un_full[:, o0:o0 + o_s]),
        })
    return in_maps, (t, i, o, o_s)


_NC_CACHE = {}


def _get_nc(t, i, o_s):
    key = (t, i, o_s)
    if key not in _NC_CACHE:
        _NC_CACHE[key] = build_nc(t, i, o_s)
    return _NC_CACHE[key]


def run(x, qweight, u, vt, trace=False, **spmd_kwargs):
    in_maps, (t, i, o, o_s) = prep_inputs(x, qweight, u, vt)
    nc = _get_nc(t, i, o_s)
    res = run_bass_kernel_spmd(
        nc, in_maps, list(range(N_CORES)), trace=trace, **spmd_kwargs
    )
    y = np.concatenate(
        [res.results[c]["y"].astype(np.float32) for c in range(N_CORES)],
        axis=1,
    )
    return y, res


def kernel(x, qweight, u, vt):
    x = np.asarray(x, dtype=np.float32)
    qweight = np.asarray(qweight)
    u = np.asarray(u, dtype=np.float32)
    vt = np.asarray(vt, dtype=np.float32)
    y, _ = run(x, qweight, u, vt, trace=False)
    return y


# revision 27
# speedup vs baseline: 1.0253x; 1.0253x over previous
"""BitStackLinear Trainium2 kernel (v2 — software-pipelined).

y = x @ w.T with w = sum_b sign_b * (u_b @ vt_b), signs bit-packed in qweight.

Strategy: column-parallel over out_features across 8 NeuronCores. Each core
builds w in bf16 (phase A) and runs y = x @ w.T (phase B), software-pipelined
so phase A for out-group og+1 runs on DVE/ACT while phase B for og runs on PE.

Phase A per 128-row i'-chunk (contraction permuted bit-plane-major so each
chunk uses one constant bit position j):
  - low-rank lr_b = 2*u_b @ vt_b via 4 concurrent K=16 row-group
    matmuls (PE), evicted PSUM->SBUF bf16 by ACT
  - DVE: AND isolates bit j as u16 {0, 2^j} (4x tensor_scalar rate), then
    one wide tensor_mul m_b = (q & 2^j) * (2^-j * 2 * lr_b) -- vtp columns
    are host-scaled by 2^-j so the power-of-2 factors cancel exactly
  - two pair-adds -> wa_chunk = sum_b m_b
The identity sign = 2*bit - 1 means w = wa - sum_b u_b@vt_b; the rank-64
correction is folded into phase B as one extra accumulating matmul per
(tq, u, og) using host-computed G = x @ vt^T (tiny) and -u.

Phase B per og: 4 token-quads, each accumulating 1 correction matmul + 32
i'-chunk matmuls into 4 PSUM banks; y evicted as bf16 (host casts to f32).
"""
import sys

for _p in ("/opt/trn_rl_repo", "/root/.axon_site/_ro/trn_rl_repo"):
    if _p not in sys.path:
        sys.path.insert(0, _p)

import numpy as np
import ml_dtypes

import concourse.bass as bass
import concourse.tile as tile
from concourse import mybir
from concourse.bass_utils import run_bass_kernel_spmd

N_CORES = 8
B = 4       # bit planes
K = 16      # low-rank
T = 2048    # tokens
I = 4096    # in_features
O = 11008   # out_features
O_S = O // N_CORES  # 1376 per core

_SLOT = 512  # psum bank free width (f32)


def _og_groups(o_s):
    """Out-feature groups, ascending width (critical-path model optimum:
    fill is priced at DVE rate on og0, tail at PE rate on the last og)."""
    if o_s == 1376:
        widths = [352, 512, 512]
    else:
        widths, rem = [], o_s
        while rem > 0:
            w = min(_SLOT, rem)
            widths.append(w)
            rem -= w
        widths.sort()
    out, o0 = [], 0
    for w in widths:
        out.append((o0, w))
        o0 += w
    return out


def build_nc(t=T, i=I, o_s=O_S):
    """Build the per-core SPMD Bass program (identical on all cores)."""
    nb = i // 8          # packed words per (b, o)
    mb_n = nb // 128     # byte-row blocks per bit plane
    nc_i = i // 128      # i'-chunks
    assert nc_i == 8 * mb_n and t % 512 == 0
    n_tq = t // 512      # token quads (4 chunks of 128 per quad)
    ogs = _og_groups(o_s)
    n_og = len(ogs)

    # phase A production order = phase B consumption order
    c_order = [j * mb_n + mb for mb in range(mb_n) for j in range(8)]

    nc = bass.Bass("TRN2", target_bir_lowering=False, debug=False)

    xt_d = nc.dram_tensor("xt", [i, t], mybir.dt.bfloat16, kind="ExternalInput")
    qt_d = nc.dram_tensor("qt", [B, nb, o_s], mybir.dt.uint16, kind="ExternalInput")
    vt_d = nc.dram_tensor("vtp", [128, i], mybir.dt.bfloat16, kind="ExternalInput")
    ut_d = nc.dram_tensor("utp", [128, o_s], mybir.dt.bfloat16, kind="ExternalInput")
    gt_d = nc.dram_tensor("gt", [64, t], mybir.dt.bfloat16, kind="ExternalInput")
    un_d = nc.dram_tensor("un", [64, o_s], mybir.dt.bfloat16, kind="ExternalInput")
    y_d = nc.dram_tensor("y", [t, o_s], mybir.dt.bfloat16, kind="ExternalOutput")

    f32 = mybir.dt.float32
    bf16 = mybir.dt.bfloat16
    u16 = mybir.dt.uint16
    AND = mybir.AluOpType.bitwise_and
    MULT = mybir.AluOpType.mult

    with tile.TileContext(nc) as tc:
        with (
            tc.tile_pool(name="const", bufs=1) as cpool,
            tc.tile_pool(name="w", bufs=1) as wpool,
            tc.tile_pool(name="q", bufs=2) as qpool,
            tc.tile_pool(name="andt", bufs=4) as andpool,
            tc.tile_pool(name="lrsb", bufs=5) as lrsbpool,
            tc.tile_pool(name="m", bufs=4) as mpool,
            tc.tile_pool(name="t", bufs=5) as tpool,
            tc.tile_pool(name="x", bufs=12) as xpool,
            tc.tile_pool(name="ysb", bufs=4) as ysbpool,
            tc.tile_pool(name="lrps", bufs=1, space="PSUM") as lrps,
            tc.tile_pool(name="yps", bufs=1, space="PSUM") as yps,
        ):
            # ---- persistent loads (q[0]/vt/ut first: phase A needs them
            # immediately; gt/un only gate the first phase-B matmul) ----
            q_sb = []

            def load_q(mb, eng=None):
                q_t = qpool.tile([128, B * o_s], u16, name=f"q{mb}",
                                 tag=f"q{mb % 2}")
                for b in range(B):
                    (eng or nc.sync).dma_start(
                        q_t[:, b * o_s:(b + 1) * o_s],
                        qt_d.ap()[b, mb * 128:(mb + 1) * 128, :],
                    )
                q_sb.append(q_t)

            # q0 first (gates the first AND), then vt/ut (first lr matmul),
            # then gt/un (first correction matmul), then the remaining q
            load_q(0)
            vt_sb = cpool.tile([128, i], bf16, tag="vt")
            nc.sync.dma_start(vt_sb[:, :512], vt_d.ap()[:, :512])
            ut_sb = cpool.tile([128, o_s], bf16, tag="ut")
            nc.sync.dma_start(ut_sb[:], ut_d.ap())
            nc.sync.dma_start(vt_sb[:, 512:], vt_d.ap()[:, 512:])
            gt_sb = cpool.tile([64, t], bf16, tag="gt")
            nc.sync.dma_start(gt_sb[:], gt_d.ap())
            un_sb = cpool.tile([64, o_s], bf16, tag="un")
            nc.sync.dma_start(un_sb[:], un_d.ap())
            for mb in range(1, mb_n):
                load_q(mb)

            w_tiles = {}

            def make_w(ogi):
                _, ow = ogs[ogi]
                w_tiles[ogi] = wpool.tile(
                    [128, nc_i * ow], bf16, name=f"w{ogi}", tag=f"w{ogi % 2}")

            def emit_a_chunk(ogi, ci):
                """Produce w_tiles[ogi][:, ci*ow:(ci+1)*ow]."""
                o0, ow = ogs[ogi]
                c = c_order[ci]
                j, mb = c // mb_n, c % mb_n
                q_t = q_sb[mb]
                # low-rank (pre-scaled): 4 concurrent K=16 row-group matmuls
                lr_ps = lrps.tile([128, B * _SLOT], f32, name="lr_ps",
                                  tag="lr_ps")
                for b in range(B):
                    nc.tensor.matmul(
                        lr_ps[:, b * _SLOT:b * _SLOT + ow],
                        vt_sb[32 * b:32 * b + K, c * 128:(c + 1) * 128],
                        ut_sb[32 * b:32 * b + K, o0:o0 + ow],
                        start=True, stop=True, tile_position=(32 * b, 0),
                    )
                # evict to packed [128, B*ow] bf16
                lr_sb = lrsbpool.tile([128, B * _SLOT], bf16, tag="lrsb")
                if ow == _SLOT:
                    nc.scalar.copy(lr_sb[:], lr_ps[:])
                else:
                    nc.scalar.copy(
                        lr_sb[:, :B * ow].rearrange("p (b w) -> p b w", b=B),
                        lr_ps[:].rearrange("p (b w) -> p b w", b=B)[:, :, :ow],
                    )
                # bit j isolated as u16 {0, 2^j} (walrus rejects mixing
                # bitwise and arith ops in one instruction, and the bitwise
                # path cannot cast dtypes; the 2^j cancels against the
                # host-side 2^-j column scaling of vtp in the multiply below,
                # which converts the u16 operand to float on its arith path)
                and_t = andpool.tile([128, B * _SLOT], u16, tag="andt")
                nc.vector.tensor_scalar(
                    and_t[:, :B * ow].rearrange("p (b w) -> p b w", b=B),
                    q_t[:].rearrange("p (b w) -> p b w", b=B)[:, :, o0:o0 + ow],
                    1 << j, None, AND,
                )
                # m_b = (q_b & 2^j) * (2^-j * 2 * lr_b) — plain 2x-rate mult
                m_t = mpool.tile([128, B * _SLOT], bf16, tag="m")
                nc.vector.tensor_mul(
                    m_t[:, :B * ow], and_t[:, :B * ow], lr_sb[:, :B * ow])
                # pair-sum the 4 planes
                t_t = tpool.tile([128, 2 * _SLOT], bf16, tag="t")
                nc.vector.tensor_add(
                    t_t[:, :2 * ow], m_t[:, :2 * ow], m_t[:, 2 * ow:4 * ow])
                nc.vector.tensor_add(
                    w_tiles[ogi][:, ci * ow:(ci + 1) * ow],
                    t_t[:, :ow], t_t[:, ow:2 * ow])

            def alloc_ysums(ogi, tq):
                o0, ow = ogs[ogi]
                ysums = [
                    yps.tile([128, _SLOT], f32, name=f"ysum{u}", tag=f"y{u}")
                    for u in range(4)
                ]
                # rank-64 sign-offset correction: y -= x @ (sum_b lr_b)^T
                for u in range(4):
                    nc.tensor.matmul(
                        ysums[u][:, :ow],
                        gt_sb[0:64, tq * 512 + u * 128:
                              tq * 512 + (u + 1) * 128],
                        un_sb[0:64, o0:o0 + ow],
                        start=True, stop=False,
                    )
                return ysums

            def emit_b_iter(ogi, tq, ci, ysums):
                o0, ow = ogs[ogi]
                c = c_order[ci]
                xt_t = xpool.tile([128, 512], bf16, tag="x")
                nc.sync.dma_start(
                    xt_t[:],
                    xt_d.ap()[c * 128:(c + 1) * 128,
                              tq * 512:(tq + 1) * 512],
                )
                for u in range(4):
                    nc.tensor.matmul(
                        ysums[u][:, :ow],
                        xt_t[:, u * 128:(u + 1) * 128],
                        w_tiles[ogi][:, ci * ow:(ci + 1) * ow],
                        start=False, stop=(ci == nc_i - 1),
                    )

            def evict_y(ogi, tq, ysums):
                o0, ow = ogs[ogi]
                for u in range(4):
                    y_sb = ysbpool.tile([128, _SLOT], bf16, tag="ysb")
                    nc.scalar.copy(y_sb[:, :ow], ysums[u][:, :ow])
                    nc.sync.dma_start(
                        y_d.ap()[(tq * 4 + u) * 128:
                                 (tq * 4 + u + 1) * 128, o0:o0 + ow],
                        y_sb[:, :ow],
                    )

            def emit_b(ogi, next_ogi, skip_tq0=False):
                """y[:, og] matmuls; interleave phase A for next_ogi."""
                ow = ogs[ogi][1]
                a_next = 0
                it = 0
                # pace A-chunk injection to the DVE chunk cadence (~2.9us)
                # so injected lr matmuls never head-block the in-order PE
                # queue: wider B iterations need fewer between chunks
                arate = 3 if ow >= 450 else 4
                for tq in range(1 if skip_tq0 else 0, n_tq):
                    ysums = alloc_ysums(ogi, tq)
                    for ci in range(nc_i):
                        emit_b_iter(ogi, tq, ci, ysums)
                        it += 1
                        if next_ogi is not None and it % arate == 0 \
                                and a_next < nc_i:
                            emit_a_chunk(next_ogi, a_next)
                            a_next += 1
                    evict_y(ogi, tq, ysums)
                # drain any unemitted A chunks (safety)
                while next_ogi is not None and a_next < nc_i:
                    emit_a_chunk(next_ogi, a_next)
                    a_next += 1

            # ---- pipelined schedule ----
            make_w(0)
            for ci in range(nc_i):
                emit_a_chunk(0, ci)
            for ogi in range(n_og):
                nxt = ogi + 1 if ogi + 1 < n_og else None
                if nxt is not None:
                    make_w(nxt)
                emit_b(ogi, nxt)

    _split_waits(nc)
    return nc


def _split_waits(nc, maxw=1):
    """This walrus build rejects instructions with more than a couple of
    sync-wait commands; move excess waits onto preceding same-engine NoOps."""
    for bb in nc.m.functions[0].blocks:
        insts = bb.instructions
        idx = 0
        while idx < len(insts):
            ins = insts[idx]
            si = ins.sync_info
            if si is not None and len(si.on_wait) > maxw:
                waits = list(si.on_wait)
                extra, keep = waits[:-maxw], waits[-maxw:]
                nops = []
                for k, wt in enumerate(extra):
                    nops.append(mybir.InstNoOp(
                        name=f"{ins.name}-wsplit{k}",
                        engine=ins.engine,
                        bass_nofuse=True,
                        sync_info=mybir.SyncInfo(on_wait=[wt], on_update=[]),
                    ))
                ins.sync_info = mybir.SyncInfo(on_wait=keep,
                                               on_update=list(si.on_update))
                for k, nop in enumerate(nops):
                    nc.register_instruction(nop, overwrite=True)
                    insts.insert(idx + k, nop)
                idx += len(nops)
            idx += 1


def prep_inputs(x, qweight, u, vt, n_cores=N_CORES):
    """Host-side layout prep + sharding. Returns (in_maps, meta)."""
    t, i = x.shape
    b_, o, k_ = u.shape
    nb = i // 8
    mb_n = nb // 128
    o_s = o // n_cores

    # x -> xt[i', t] bf16 with i' = j*(i/8) + m  (j-major bit-plane order)
    xt = np.ascontiguousarray(
        x.T.reshape(nb, 8, t).transpose(1, 0, 2).reshape(i, t)
    ).astype(ml_dtypes.bfloat16)

    # qweight -> qt[b, m, o] uint16 (byte-transposed; u16 ops hit the DVE
    # 16-bit packed mode, 2x the u8 rate)
    qt = np.ascontiguousarray(
        qweight.astype(np.uint16).reshape(b_, o, nb).transpose(0, 2, 1)
    )

    # vt -> permuted into i' order, stacked into PE row groups [128, i], with
    # per-chunk 2^-j column scaling (cancelled on-chip by the {0, 2^j} AND
    # result); the sign-identity factor 2 is folded into utp.
    vtp = vt.reshape(b_, k_, nb, 8).transpose(0, 1, 3, 2).reshape(b_, k_, i)
    vt_stack = np.zeros((128, i), np.float32)
    for b in range(b_):
        vt_stack[32 * b:32 * b + k_, :] = vtp[b]
    jcol = np.arange(i) // (i // 8)  # bit position j per i' column
    vt_stack *= np.exp2(-jcol)[None, :].astype(np.float32)
    vt_stack = vt_stack.astype(ml_dtypes.bfloat16)

    # u -> 2 * u^T stacked [128, o], bf16
    ut_full = np.zeros((128, o), np.float32)
    for b in range(b_):
        ut_full[32 * b:32 * b + k_, :] = 2.0 * u[b].T
    ut_full = ut_full.astype(ml_dtypes.bfloat16)

    # G = x @ vt_b^T per plane, stacked [64, t]; un = -u^T stacked [64, o]
    gt_full = np.zeros((64, t), np.float32)
    un_full = np.zeros((64, o), np.float32)
    for b in range(b_):
        gt_full[16 * b:16 * b + k_, :] = (x @ vt[b].T).T
        un_full[16 * b:16 * b + k_, :] = -u[b].T
    gt_full = gt_full.astype(ml_dtypes.bfloat16)
    un_full = un_full.astype(ml_dtypes.bfloat16)

    in_maps = []
    for core in range(n_cores):
        o0 = core * o_s
        in_maps.append({
            "xt": xt,
            "qt": np.ascontiguousarray(qt[:, :, o0:o0 + o_s]),
            "vtp": vt_stack,
            "utp": np.ascontiguousarray(ut_full[:, o0:o0 + o_s]),
            "gt": gt_full,
            "un": np.ascontiguousarray(un_full[:, o0:o0 + o_s]),
        })
    return in_maps, (t, i, o, o_s)


_NC_CACHE = {}


def _get_nc(t, i, o_s):
    key = (t, i, o_s)
    if key not in _NC_CACHE:
        _NC_CACHE[key] = build_nc(t, i, o_s)
    return _NC_CACHE[key]


def run(x, qweight, u, vt, trace=False, **spmd_kwargs):
    in_maps, (t, i, o, o_s) = prep_inputs(x, qweight, u, vt)
    nc = _get_nc(t, i, o_s)
    res = run_bass_kernel_spmd(
        nc, in_maps, list(range(N_CORES)), trace=trace, **spmd_kwargs
    )
    y = np.concatenate(
        [res.results[c]["y"].astype(np.float32) for c in range(N_CORES)],
        axis=1,
    )
    return y, res


def kernel(x, qweight, u, vt):
    x = np.asarray(x, dtype=np.float32)
    qweight = np.asarray(qweight)
    u = np.asarray(u, dtype=np.float32)
    vt = np.asarray(vt, dtype=np.float32)
    y, _ = run(x, qweight, u, vt, trace=False)
    return y
